# revision 71
# baseline (speedup 1.0000x reference)
"""Trainium2 Bass kernel for nn_BASE_49821620633700 (sparse_attention).

Pipeline (replicated on all 8 NeuronCores; host reads core 0):
  SE squeeze-excite (pool from channel-major x, free-dim reduce)
  R = out32^T = x^T * sigmoid-scale   (position-major, via K=1 broadcast matmul)
  gaussian non-local:  G = gus @ R    (TensorE, bf16, f32 accum)
  patch attention:     3x3 neighbor gather (shifted DMA) -> per-position
                       dot-product scores (DVE fused mul+reduce) -> softmax
                       -> weighted neighbor sum (ACT/DVE/GPSIMD split)
  down conv in CHANNEL-major layout:  O = W^T.T @ F  (TensorE, bf16) so
  InstanceNorm stats are free-dim reduces and the normalize is a single
  fused per-partition tensor_scalar; LeakyReLU = max(x, 0.2x).

Sharding decision: measured 8-rank collective costs on this fleet are
65-138us (vs ~15us whole-kernel floor and ~30us total compute), so any
cross-core exchange erases the 8x compute win.  The kernel runs fully
replicated (zero collectives).
"""
import sys

if "/opt/trn_rl_repo" not in sys.path:
    sys.path.insert(0, "/opt/trn_rl_repo")

import numpy as np
import concourse.bass as bass
import concourse.mybir as mybir
from concourse import tile
from concourse.bass_types import AP
from concourse.bass_utils import run_bass_kernel_spmd

F32 = mybir.dt.float32
BF16 = mybir.dt.bfloat16
AF = mybir.ActivationFunctionType
ALU = mybir.AluOpType

H = W = 32
HW = H * W          # 1024 positions
C = 512             # channels
R_SE = C // 16      # 32
EPS = 1e-5
NT = HW // 128      # 8 position tiles of 128
KC = C // 128       # 4 channel chunks of 128
OFFS = [(dy, dx) for dy in (-1, 0, 1) for dx in (-1, 0, 1)]
MASKVAL = -100.0 * C  # pre-scale (raw-sum domain) invalid-neighbor score -> exp == 0

NCORES = 8


def gussin_np(v=1.5, n=32):
    d = (np.arange(n)[:, None] - np.arange(n)[None, :]).astype(np.float64) ** 2
    g = np.exp(-(d[:, None, :, None] + d[None, :, None, :]) / (2.0 * v * v)) / (
        2.0 * np.pi * v * v
    )
    g = g.reshape(n * n, n, n)
    return (g / g.sum((-1, -2), keepdims=True)).astype(np.float32)


def _bf16(a):
    import ml_dtypes

    return a.astype(ml_dtypes.bfloat16)


def prep_inputs(x, se_w1, se_b1, se_w2, se_b2, down_w):
    x = np.asarray(x, np.float32)
    xn = np.ascontiguousarray(x.reshape(C, HW))                        # (512, 1024)
    # x^T with 64 zero guard rows on each side: the unscaled R surrogate (the
    # SE sigmoid scale commutes through the value/down matmuls as a
    # per-output-column factor and is applied once at the A+B merge).
    rdpad = np.zeros((HW + 128, C), np.float32)
    rdpad[64:64 + HW] = xn.T
    gus = gussin_np(1.5, H).reshape(HW, HW)                            # (1024,1024)
    w1 = np.asarray(down_w, np.float32)[:, :C]
    # fold the first half of the down conv through the (constant) gaussian:
    # O_A[o, (h, gc)] = sum_q (w1 @ gus[h::2])[o, q] * R[q, gc]
    m0T = np.ascontiguousarray((w1 @ gus[0::2]).T)                     # (1024, 512)
    m1T = np.ascontiguousarray((w1 @ gus[1::2]).T)
    w2T = np.ascontiguousarray(np.asarray(down_w, np.float32)[:, C:].T)
    # fold the 1/HW pooling mean into the first SE matmul
    se_w1T = np.ascontiguousarray(np.asarray(se_w1, np.float32).T) / HW  # (512, 32)
    se_w2T = np.ascontiguousarray(np.asarray(se_w2, np.float32).T)       # (32, 512)
    b1 = np.asarray(se_b1, np.float32).reshape(R_SE, 1)
    b2 = np.asarray(se_b2, np.float32).reshape(1, C)

    # Banded-attention mask for one 64-position block (u-major: p = 32u + c):
    # band columns are 4 image rows x 32 cols; position (u, c) attends band
    # rows {u, u+1, u+2} and cols {c-1, c, c+1}.  Uniform across blocks; the
    # zero guard rows/cols reproduce the reference's zero-padding semantics.
    u = np.arange(2)[:, None, None, None]
    c = np.arange(W)[None, :, None, None]
    i = np.arange(4)[None, None, :, None]
    qc = np.arange(W)[None, None, None, :]
    sel = (i >= u) & (i <= u + 2) & (np.abs(qc - c) <= 1)
    maskbig = np.where(sel, 0.0, MASKVAL).reshape(64, 128).astype(np.float32)
    # corner columns lose one dx slot per dy row vs the reference's 9-window:
    # add the missing exp(0)=1 terms to the softmax denominator.
    corr = np.where((np.arange(W) % W) % 31 == 0, 3.0, 0.0)
    corr = np.tile(corr, 2).reshape(64, 1).astype(np.float32)
    # Permutation matrix used in place of the transpose identity so the
    # value-matmul output lands in parity-major (position-pair) order, which
    # is the down conv's flat-view pairing: new row 32*par+16*u+cp takes old
    # row 32*u+2*cp+par.
    pu, pp, pc = np.meshgrid(np.arange(2), np.arange(2), np.arange(16), indexing="ij")
    old_row = (32 * pu + 2 * pc + pp).transpose(1, 0, 2).reshape(64)
    permmat = np.zeros((64, 64), np.float32)
    permmat[old_row, np.arange(64)] = 1.0

    return {
        "xn": _bf16(xn),
        "rdpad": _bf16(rdpad),
        "m0T": _bf16(m0T),
        "m1T": _bf16(m1T),
        "w2T": _bf16(w2T),
        "se_w1T": se_w1T,
        "se_w2T": se_w2T,
        "se_b1": b1,
        "se_b2": b2,
        "se_b2c": np.ascontiguousarray(b2.reshape(C, 1)),
        "maskbig": maskbig,
        "corr": corr,
        "permmat": permmat,
    }


DEBUG_DUMPS = False


def build_nc():
    nc = bass.Bass(target_bir_lowering=False, debug=False)

    xn_d = nc.declare_dram_parameter("xn", [C, HW], BF16, isOutput=False)
    rdpad_d = nc.declare_dram_parameter("rdpad", [HW + 128, C], BF16, isOutput=False)
    m0T_d = nc.declare_dram_parameter("m0T", [HW, C], BF16, isOutput=False)
    m1T_d = nc.declare_dram_parameter("m1T", [HW, C], BF16, isOutput=False)
    w2T_d = nc.declare_dram_parameter("w2T", [C, C], BF16, isOutput=False)
    se_w1T_d = nc.declare_dram_parameter("se_w1T", [C, R_SE], F32, isOutput=False)
    se_w2T_d = nc.declare_dram_parameter("se_w2T", [R_SE, C], F32, isOutput=False)
    se_b1_d = nc.declare_dram_parameter("se_b1", [R_SE, 1], F32, isOutput=False)
    se_b2_d = nc.declare_dram_parameter("se_b2", [1, C], F32, isOutput=False)
    se_b2c_d = nc.declare_dram_parameter("se_b2c", [C, 1], F32, isOutput=False)
    maskbig_d = nc.declare_dram_parameter("maskbig", [64, 128], F32, isOutput=False)
    corr_d = nc.declare_dram_parameter("corr", [64, 1], F32, isOutput=False)
    permmat_d = nc.declare_dram_parameter("permmat", [64, 64], F32, isOutput=False)
    out_d = nc.declare_dram_parameter("out", [C, HW], F32, isOutput=True)

    GP = 64
    Rd = rdpad_d
    # patch-attention output, stored pre-paired for the down conv flat view:
    # Cd2[h, r, c] = out_csa[position 2r+h, channel c]
    Cd2 = nc.dram_tensor("C_dram", [2, C, C], BF16)
    dbg = {}
    if DEBUG_DUMPS:
        for nm in ("dbgR", "dbgS", "dbgG", "dbgC"):
            dbg[nm] = nc.declare_dram_parameter(nm, [HW, C], BF16, isOutput=True)
        dbg["dbgStat"] = nc.declare_dram_parameter(
            "dbgStat", [128, 4 * KC], F32, isOutput=True
        )
        dbg["dbgA"] = nc.declare_dram_parameter("dbgA", [HW, 9], F32, isOutput=True)
        dbg["dbgSc"] = nc.declare_dram_parameter("dbgSc", [HW, 9], F32, isOutput=True)

    with tile.TileContext(nc) as tc:
        with (
            tc.tile_pool(name="const", bufs=1) as constp,
            tc.tile_pool(name="big", bufs=1) as bigp,
            tc.tile_pool(name="work", bufs=2) as workp,
            tc.tile_pool(name="nbr", bufs=2) as nbrp,
        ):
            # ---------- constants ----------
            eps_sb = constp.tile([128, 1], F32, tag="eps", name="eps_sb")
            nc.gpsimd.memset(eps_sb[:], EPS)
            zero_col = constp.tile([128, 1], F32, tag="zeroc", name="zero_col")
            nc.gpsimd.memset(zero_col[:], 0.0)
            ones_row = constp.tile([1, 128], F32, tag="ones_row", name="ones_row")
            nc.gpsimd.memset(ones_row[:], 1.0)
            b1_sb = constp.tile([R_SE, 1], F32, tag="b1", name="b1_sb")
            nc.sync.dma_start(out=b1_sb[:], in_=se_b1_d[:])
            b2_sb = constp.tile([1, C], F32, tag="b2", name="b2_sb")
            nc.sync.dma_start(out=b2_sb[:], in_=se_b2_d[:])
            sw1 = []
            for k in range(KC):
                t_ = constp.tile([128, R_SE], F32, tag=f"sw1_{k}", name=f"sw1_{k}")
                nc.sync.dma_start(out=t_[:], in_=se_w1T_d[128 * k:128 * (k + 1), :])
                sw1.append(t_)
            sw2 = constp.tile([R_SE, C], F32, tag="sw2", name="sw2")
            nc.sync.dma_start(out=sw2[:], in_=se_w2T_d[:])
            b2c_sb = constp.tile([C // KC, KC], F32, tag="b2c", name="b2c_sb")
            nc.sync.dma_start(
                out=b2c_sb[:], in_=se_b2c_d.rearrange("(k p) o -> p (k o)", k=KC)
            )
            maskbig_sb = constp.tile([64, 128], F32, tag="maskbig", name="maskbig_sb")
            nc.sync.dma_start(out=maskbig_sb[:], in_=maskbig_d[:])
            corr_sb = constp.tile([64, 1], F32, tag="corr", name="corr_sb")
            nc.sync.dma_start(out=corr_sb[:], in_=corr_d[:])
            perm_sb = constp.tile([64, 64], F32, tag="perm64", name="perm_sb")
            nc.sync.dma_start(out=perm_sb[:], in_=permmat_d[:])

            xn_sb, xt_sb, w2_sb = [], [], []
            for k in range(KC):
                t_ = bigp.tile([128, HW], BF16, tag=f"xn{k}", name=f"xn{k}")
                nc.sync.dma_start(out=t_[:], in_=xn_d[128 * k:128 * (k + 1), :])
                xn_sb.append(t_)
            for t in range(NT):
                t_ = bigp.tile([128, C], BF16, tag=f"xt{t}", name=f"xt{t}")
                nc.sync.dma_start(
                    out=t_[:], in_=rdpad_d[GP + 128 * t:GP + 128 * (t + 1), :]
                )
                xt_sb.append(t_)
            # folded gaussian-down weights + w2 stream in on the scalar queue
            mT_sb = {0: [], 1: []}
            for h, md in ((0, m0T_d), (1, m1T_d)):
                for k in range(NT):
                    t_ = bigp.tile([128, C], BF16, tag=f"m{h}_{k}", name=f"m{h}_{k}")
                    nc.scalar.dma_start(out=t_[:], in_=md[128 * k:128 * (k + 1), :])
                    mT_sb[h].append(t_)
            for k in range(KC):
                t2_ = bigp.tile([128, C], BF16, tag=f"w2_{k}", name=f"w2_{k}")
                nc.scalar.dma_start(out=t2_[:], in_=w2T_d[128 * k:128 * (k + 1), :])
                w2_sb.append(t2_)

            # channel-major sigmoid(out32) with 32-col zero guards on each side
            scm_sb = [
                bigp.tile([128, HW + 64], BF16, tag=f"scm{k}", name=f"scm{k}")
                for k in range(KC)
            ]
            # down-conv output, channel-major (4 x (128ch, 1024pos))
            o_sb = [
                bigp.tile([128, HW], F32, tag=f"o{m}", name=f"o{m}") for m in range(KC)
            ]

            # ---------- SE layer (scoped PSUM) ----------
            with tc.tile_pool(name="ps_se", bufs=1, space="PSUM") as pse:
                ysum = workp.tile([128, KC], F32, tag="ysum", name="ysum")
                for k in range(KC):
                    nc.vector.reduce_sum(
                        ysum[:, k:k + 1], xn_sb[k][:], axis=mybir.AxisListType.X
                    )
                y1_ps = pse.tile([R_SE, 1], F32, tag="y1", name="y1_ps")
                for k in range(KC):
                    nc.tensor.matmul(
                        y1_ps[:],
                        sw1[k][:],
                        ysum[:, k:k + 1],
                        start=(k == 0),
                        stop=(k == KC - 1),
                    )
                y1_sb = workp.tile([R_SE, 1], F32, tag="y1_sb", name="y1_sb")
                nc.scalar.activation(y1_sb[:], y1_ps[:], AF.Relu, bias=b1_sb[:])

                y2_ps = pse.tile([1, C], F32, tag="y2", name="y2_ps")
                nc.tensor.matmul(y2_ps[:], y1_sb[:], sw2[:], start=True, stop=True)
                y2pb_sb = workp.tile([1, C], F32, tag="y2pb", name="y2pb_sb")
                nc.vector.tensor_tensor(
                    out=y2pb_sb[:], in0=y2_ps[:], in1=b2_sb[:], op=ALU.add
                )
                y2_sb = workp.tile([1, C], F32, tag="y2s", name="y2_sb")
                nc.scalar.activation(y2_sb[:], y2pb_sb[:], AF.Sigmoid)

                ybc_ps = pse.tile([128, C], F32, tag="ybc", name="ybc_ps")
                nc.tensor.matmul(
                    ybc_ps[:], ones_row[:], y2_sb[:], start=True, stop=True
                )
                ybc_sb = bigp.tile([128, C], F32, tag="ybc_sb", name="ybc_sb")
                nc.vector.tensor_copy(ybc_sb[:], ybc_ps[:])
                # y2 as per-channel column scalars (128, KC)
                y2c_ps = pse.tile([128, KC], F32, tag="y2c", name="y2c_ps")
                for k in range(KC):
                    nc.tensor.matmul(
                        y2c_ps[:, k:k + 1],
                        sw2[:, 128 * k:128 * (k + 1)],
                        y1_sb[:],
                        start=True,
                        stop=True,
                    )
                y2cb = workp.tile([128, KC], F32, tag="y2cb", name="y2cb")
                nc.vector.tensor_tensor(
                    out=y2cb[:], in0=y2c_ps[:], in1=b2c_sb[:], op=ALU.add
                )
                y2c_sb = workp.tile([128, KC], F32, tag="y2cs", name="y2c_sb")
                nc.scalar.activation(y2c_sb[:], y2cb[:], AF.Sigmoid)

                # ---------- S (channel-major, scaled) ----------
                for k in range(KC):
                    nc.gpsimd.memset(scm_sb[k][:, 0:32], 0.0)
                    nc.gpsimd.memset(scm_sb[k][:, 32 + HW:64 + HW], 0.0)
                    rcmk = workp.tile([128, HW], F32, tag="rcmk", name=f"rcmk{k}")
                    nc.scalar.activation(
                        rcmk[:], xn_sb[k][:], AF.Copy, scale=y2c_sb[:, k:k + 1]
                    )
                    nc.scalar.activation(
                        scm_sb[k][:, 32:32 + HW], rcmk[:], AF.Sigmoid
                    )

            # ---------- main PSUM pool ----------
            with tc.tile_pool(name="ps_main", bufs=2, space="PSUM") as psmain:
                # patch attention as banded matmuls: 16 blocks of 64 query
                # positions (2 image rows) x 128 band positions (4 image rows).
                # scores = S^T S on TensorE (channel-major S with zero guard
                # cols), masked softmax on ACT/DVE, weighted value sum as a
                # second matmul against position-major R (zero guard rows).
                oa_sb = {}

                def emit_oa(m, h):
                    oa_ps = psmain.tile([128, C], F32, tag="g_ps", name=f"oa{m}_{h}")
                    for k in range(NT):
                        nc.tensor.matmul(
                            oa_ps[:],
                            mT_sb[h][k][:, 128 * m:128 * (m + 1)],
                            xt_sb[k][:],
                            start=(k == 0),
                            stop=(k == NT - 1),
                        )
                    oa = workp.tile([128, C], F32, tag=f"oa{m}_{h}", name=f"oa{m}_{h}")
                    nc.vector.tensor_copy(oa[:], oa_ps[:])
                    oa_sb[(m, h)] = oa

                for s in range(16):
                    sc_ps = psmain.tile([64, 128], F32, tag="sc_ps", name=f"sc_ps{s}")
                    for k in range(KC):
                        nc.tensor.matmul(
                            lhsT=scm_sb[k][:, 32 + 64 * s:32 + 64 * s + 64],
                            rhs=scm_sb[k][:, 64 * s:64 * s + 128],
                            out=sc_ps[:],
                            start=(k == 0),
                            stop=(k == KC - 1),
                        )
                    if s % 2 == 0:
                        g = s // 2
                        emit_oa(g % KC, g // KC)
                    sc2 = workp.tile([64, 128], F32, tag="sc2", name=f"sc2_{s}")
                    nc.vector.tensor_tensor(
                        out=sc2[:], in0=sc_ps[:], in1=maskbig_sb[:], op=ALU.add
                    )
                    e = workp.tile([64, 128], F32, tag="e", name=f"e{s}")
                    nc.scalar.activation(e[:], sc2[:], AF.Exp, scale=1.0 / C)
                    esum0 = workp.tile([64, 1], F32, tag="esum0", name=f"es0_{s}")
                    nc.vector.reduce_sum(esum0[:], e[:], axis=mybir.AxisListType.X)
                    esum = workp.tile([64, 1], F32, tag="esum", name=f"es{s}")
                    nc.vector.tensor_tensor(
                        out=esum[:], in0=esum0[:], in1=corr_sb[:], op=ALU.add
                    )
                    rinv = workp.tile([64, 1], F32, tag="rinv", name=f"ri{s}")
                    nc.vector.reciprocal(rinv[:], esum[:])
                    wn = workp.tile([64, 128], F32, tag="wn", name=f"wn{s}")
                    nc.vector.tensor_scalar_mul(wn[:], e[:], rinv[:])

                    eT_ps = psmain.tile([128, 64], F32, tag="eT_ps", bufs=1, name=f"eT{s}")
                    nc.tensor.transpose(eT_ps[:], wn[:], perm_sb[:])
                    eT_sb = workp.tile([128, 64], BF16, tag="eT_sb", name=f"eTs{s}")
                    nc.vector.tensor_copy(eT_sb[:], eT_ps[:])

                    rband = nbrp.tile([128, C], BF16, tag="rband", name=f"rband{s}")
                    nc.sync.dma_start(
                        out=rband[:], in_=Rd[GP + 64 * s - 32:GP + 64 * s + 96, :]
                    )
                    cc_ps = psmain.tile([64, C], F32, tag="cc_ps", bufs=1, name=f"cc{s}")
                    nc.tensor.matmul(
                        cc_ps[:], eT_sb[:], rband[:], start=True, stop=True
                    )
                    c_bf = workp.tile([64, C], BF16, tag="c_bf", name=f"cb{s}")
                    nc.scalar.activation(c_bf[:], cc_ps[:], AF.Copy)
                    for par in range(2):
                        nc.gpsimd.dma_start(
                            out=Cd2[par, 32 * s:32 * (s + 1), :],
                            in_=c_bf[32 * par:32 * par + 32, :],
                        )

                # ---- B feature tiles read contiguously from pre-paired Cd2 ----
                f_sb = {}
                for h in range(2):
                    for tt in range(4):
                        ft = bigp.tile(
                            [128, C], BF16, tag=f"f{h}_{tt}", name=f"f{h}_{tt}"
                        )
                        nc.gpsimd.dma_start(
                            out=ft[:],
                            in_=Cd2[h, 128 * tt:128 * (tt + 1), :],
                        )
                        f_sb[(h, tt)] = ft

                # ---- down conv (channel-major) + stats ----
                sums = workp.tile([128, 2 * KC], F32, tag="sums", name="sums")
                sqs = workp.tile([128, 2 * KC], F32, tag="sqs", name="sqs")
                for m in range(KC):
                    for h in range(2):
                        o_ps = psmain.tile([128, C], F32, tag="o_ps", name=f"o_ps{m}_{h}")
                        for tt in range(4):
                            nc.tensor.matmul(
                                o_ps[:],
                                w2_sb[tt][:, 128 * m:128 * (m + 1)],
                                f_sb[(h, tt)][:],
                                start=(tt == 0),
                                stop=(tt == 3),
                            )
                        half = o_sb[m][:, C * h:C * (h + 1)]
                        ab = workp.tile([128, C], F32, tag="abm", name=f"abm{m}_{h}")
                        nc.vector.tensor_tensor(
                            out=ab[:], in0=oa_sb[(m, h)][:], in1=o_ps[:], op=ALU.add
                        )
                        nc.vector.scalar_tensor_tensor(
                            out=half,
                            in0=ab[:],
                            scalar=1.0,
                            in1=ybc_sb[:],
                            op0=ALU.mult,
                            op1=ALU.mult,
                            accum_out=sums[:, 2 * m + h:2 * m + h + 1],
                        )
                        sqjunk = workp.tile([128, C], F32, tag="sqjunk", name=f"sq{m}_{h}")
                        nc.scalar.activation(
                            sqjunk[:], half, AF.Square,
                            accum_out=sqs[:, 2 * m + h:2 * m + h + 1],
                        )

                # ---- instance norm + leaky relu (per-partition scalars) ----
                for m in range(KC):
                    msum = workp.tile([128, 1], F32, tag="msum", name=f"msum{m}")
                    nc.vector.tensor_tensor(
                        out=msum[:], in0=sums[:, 2 * m:2 * m + 1],
                        in1=sums[:, 2 * m + 1:2 * m + 2], op=ALU.add,
                    )
                    mean = workp.tile([128, 1], F32, tag="meanc", name=f"mean{m}")
                    nc.vector.tensor_scalar_mul(mean[:], msum[:], 1.0 / HW)
                    qsum = workp.tile([128, 1], F32, tag="qsum", name=f"qsum{m}")
                    nc.vector.tensor_tensor(
                        out=qsum[:], in0=sqs[:, 2 * m:2 * m + 1],
                        in1=sqs[:, 2 * m + 1:2 * m + 2], op=ALU.add,
                    )
                    esqm = workp.tile([128, 1], F32, tag="esqm", name=f"esq{m}")
                    nc.vector.tensor_scalar_mul(esqm[:], qsum[:], 1.0 / HW)
                    msq = workp.tile([128, 1], F32, tag="msqc", name=f"msq{m}")
                    nc.vector.tensor_tensor(
                        out=msq[:], in0=mean[:], in1=mean[:], op=ALU.mult
                    )
                    var = workp.tile([128, 1], F32, tag="varc", name=f"var{m}")
                    nc.vector.tensor_tensor(
                        out=var[:], in0=esqm[:], in1=msq[:], op=ALU.subtract
                    )
                    std = workp.tile([128, 1], F32, tag="stdc", name=f"std{m}")
                    nc.scalar.activation(std[:], var[:], AF.Sqrt, bias=eps_sb[:])
                    rstd = workp.tile([128, 1], F32, tag="rstdc", name=f"rstd{m}")
                    nc.vector.reciprocal(rstd[:], std[:])
                    nmr = workp.tile([128, 1], F32, tag="nmr", name=f"nmr{m}")
                    nc.vector.tensor_tensor(
                        out=nmr[:], in0=mean[:], in1=rstd[:], op=ALU.mult
                    )
                    nmrn = workp.tile([128, 1], F32, tag="nmrn", name=f"nmrn{m}")
                    nc.vector.tensor_scalar_mul(nmrn[:], nmr[:], -1.0)

                    t2 = workp.tile([128, HW], F32, tag="t2", bufs=1, name=f"t2_{m}")
                    nc.vector.tensor_scalar(
                        out=t2[:],
                        in0=o_sb[m][:],
                        scalar1=rstd[:],
                        scalar2=nmrn[:],
                        op0=ALU.mult,
                        op1=ALU.add,
                    )
                    t3 = workp.tile([128, HW], F32, tag="t3", bufs=1, name=f"t3_{m}")
                    nc.scalar.activation(t3[:], t2[:], AF.Copy, scale=0.2)
                    ot = workp.tile([128, HW], F32, tag="ot", bufs=1, name=f"ot{m}")
                    nc.vector.tensor_tensor(
                        out=ot[:], in0=t2[:], in1=t3[:], op=ALU.max
                    )
                    nc.sync.dma_start(out=out_d[128 * m:128 * (m + 1), :], in_=ot[:])

                if DEBUG_DUMPS:
                    nc.sync.dma_start(out=dbg["dbgR"][:], in_=Rd[GP:GP + HW, :])
                    nc.sync.dma_start(out=dbg["dbgStat"][:, 0:2 * KC], in_=sums[:])
                    nc.sync.dma_start(out=dbg["dbgStat"][:, 2 * KC:4 * KC], in_=sqs[:])

    return nc


def _split_drain_waits(nc, keep=1):
    """This walrus build allows at most 1 sync wait per instruction; hoist the
    extras onto preceding NoOps on the same engine."""
    n = 0
    for f in nc.m.functions:
        for bb in f.blocks:
            newlist = []
            for ins in bb.instructions:
                si = getattr(ins, "sync_info", None)
                if (
                    si is not None
                    and si.on_wait
                    and len(si.on_wait) > keep
                ):
                    waits = list(si.on_wait)
                    for w in waits[:-keep]:
                        nop = mybir.InstNoOp(name=f"I-dw{n}", ins=[], outs=[])
                        n += 1
                        nop.engine = ins.engine
                        nop.sync_info = mybir.SyncInfo(on_wait=[w], on_update=[])
                        newlist.append(nop)
                    si.on_wait = waits[-keep:]
                newlist.append(ins)
            bb.instructions = newlist
    return n


_BUILT = None


def get_built():
    global _BUILT
    if _BUILT is None:
        nc = build_nc()
        _split_drain_waits(nc)
        _BUILT = nc
    return _BUILT


def kernel(x, se_w1, se_b1, se_w2, se_b2, down_w, _trace=False):
    ins = prep_inputs(x, se_w1, se_b1, se_w2, se_b2, down_w)
    nc = get_built()
    in_maps = [dict(ins) for _ in range(NCORES)]
    res = run_bass_kernel_spmd(nc, in_maps, list(range(NCORES)), trace=_trace)
    out = np.asarray(res.results[0]["out"], np.float32)  # (512, 1024) ch-major
    full = np.ascontiguousarray(out).reshape(1, C, H, W)
    if _trace:
        return full, res
    return full
